# revision 1
# baseline (speedup 1.0000x reference)
"""Deformable conv2d (torchvision semantics: stride=1, pad=0, dil=1,
offset_groups=1, no mask/bias) on 8 TRN2 NeuronCores via Bass/Tile.

Hardcoded shapes: input [16,64,66,66] f32, offset [16,18,64,64] f32,
weight [64,64,3,3] f32 -> out [16,64,64,64] f32.

Sharding: data-parallel over batch; core i handles images (2i, 2i+1).

Per-core scheme (per image; SBUF partitions = (x-side h in {0,1}, channel c)):
  - patch table T[p, ty*68+tx, jy] = I[c(p), ty-1+jy, tx-1], same content in
    both partition halves; the upper half gathers with idx+1 so it reads the
    x0+1 column. OOB entries are garbage but always weight-masked.
  - GPSIMD ap_gather (d=2) fetches per tap/pixel the 4 bilinear corners.
  - bilinear corner weights (validity folded in) are computed compactly
    (pixels spread across partitions), then broadcast to the (h,c) layout
    with a K=2 indicator matmul into PSUM.
  - DVE: S = G * W (f32r out); TensorE: per tap 2 matmuls (jy=0/1)
    contracting (h,c)=128 with lhsT=[wk;wk], accumulating 9 taps in PSUM.
"""

import sys

sys.path.insert(0, "/opt/trn_rl_repo")

import numpy as np

import concourse.bacc as bacc
import concourse.mybir as mybir
import concourse.tile as tile

F32 = mybir.dt.float32
F32R = mybir.dt.float32r
BF16 = mybir.dt.bfloat16
I16 = mybir.dt.int16
I32 = mybir.dt.int32

N, CIN, COUT = 16, 64, 64
HIN, WIN = 66, 66
KH, KW = 3, 3
HO, WO = 64, 64
K = KH * KW
NPX = HO * WO
NCORES = 8

TE = 68
NE = TE * TE  # 4624
CHUNK = 512
NCHUNK = NPX // CHUNK  # 8
KF = K * 64  # 576
WF = 2 * KF  # 1152

VAL_DT = BF16


def _alu(name):
    return getattr(mybir.AluOpType, name)


def build_bass():
    nc = bacc.Bacc("TRN2", target_bir_lowering=False, debug=False,
                   num_devices=NCORES)

    din = {}
    for nm, shp in [
        ("imga", [128, HIN * WIN]), ("imgb", [128, HIN * WIN]),
        ("wmat", [128, K * 64]), ("ind2", [32, 2048]), ("hsel", [128, 1]),
        ("dyc", [128, KF]), ("dxc", [128, KF]),
        ("byc", [128, KF]), ("bxc", [128, KF]),
        ("dyt", [128, WF]), ("dxt", [128, WF]),
        ("byt", [128, WF]), ("bxt", [128, WF]),
    ]:
        din[nm] = nc.dram_tensor(nm, shp, F32, kind="ExternalInput")
    out_d = nc.dram_tensor("out128", [128, NPX], F32, kind="ExternalOutput")
    stage_d = nc.dram_tensor("wcstage", [128, 4 * KF], F32R)
    istage = [nc.dram_tensor(f"idxstage{i}", [128, KF], I16) for i in range(2)]

    with tile.TileContext(nc) as tc:
        with tc.tile_pool(name="cst", bufs=1) as cpool:
            # persistent tiles
            wmat = cpool.tile([128, K * 64], F32R, name="wmat_r")
            ind2 = cpool.tile([32, 2048], F32R, name="ind2_r")
            hsel = cpool.tile([128, 1], F32, name="hsel_t")
            nc.sync.dma_start(hsel[:], din["hsel"].ap())
            idxws = [cpool.tile([128, 4 * KF], I16, tag=f"idxw{i}", name=f"idxw{i}")
                     for i in range(2)]
            wc32 = cpool.tile([32, 16 * KF], F32R, name="wc32")

            # ---------- phase 1: index + weight pipelines ----------
            with tc.tile_pool(name="pipe", bufs=1) as tp:
                wmat_f = tp.tile([128, K * 64], F32, tag="wmf", name="wmat_f")
                nc.sync.dma_start(wmat_f[:], din["wmat"].ap())
                nc.vector.tensor_copy(wmat[:], wmat_f[:])
                ind2_f = tp.tile([32, 2048], F32, tag="i2f", name="ind2_f")
                nc.sync.dma_start(ind2_f[:], din["ind2"].ap())
                nc.vector.tensor_copy(ind2[:], ind2_f[:])

                _cnt = [0]

                def _nm(tg):
                    _cnt[0] += 1
                    return f"{tg}_{_cnt[0]}"

                def ld(nm, nf, tg):
                    t = tp.tile([128, nf], F32, tag=tg, name=_nm(nm))
                    nc.sync.dma_start(t[:], din[nm].ap())
                    return t

                def tmp(tg, nf=WF, dt=F32):
                    return tp.tile([128, nf], dt, tag=tg, name=_nm(tg))

                def floor_(x, out, nf):
                    ti = tmp("fl_i", nf, I32)
                    nc.vector.tensor_copy(ti[:], x[:])
                    tf = tmp("fl_f", nf)
                    nc.vector.tensor_copy(tf[:], ti[:])
                    co = tmp("fl_c", nf)
                    nc.vector.tensor_tensor(co[:], tf[:], x[:], _alu("is_gt"))
                    nc.vector.tensor_sub(out[:], tf[:], co[:])

                # --- idx pipeline (compact [128, KF]) ---
                byc = ld("byc", KF, "byc")
                dyc = ld("dyc", KF, "dyc")
                pyc = tmp("p1", KF)
                nc.vector.tensor_add(pyc[:], byc[:], dyc[:])
                y0c = tmp("v1", KF)
                floor_(pyc, y0c, KF)
                tyc = tmp("p2", KF)
                nc.vector.tensor_scalar(tyc[:], y0c[:], 1.0, 67.0,
                                        _alu("add"), _alu("min"))
                tyc2 = tmp("p3", KF)
                nc.vector.tensor_scalar(tyc2[:], tyc[:], 0.0, float(TE),
                                        _alu("max"), _alu("mult"))
                bxc = ld("bxc", KF, "byc")
                dxc = ld("dxc", KF, "dyc")
                pxc = tmp("p1", KF)
                nc.vector.tensor_add(pxc[:], bxc[:], dxc[:])
                x0c = tmp("v2", KF)
                floor_(pxc, x0c, KF)
                txc = tmp("p4", KF)
                nc.vector.tensor_scalar(txc[:], x0c[:], 1.0, 67.0,
                                        _alu("add"), _alu("min"))
                txc2 = tmp("p5", KF)
                nc.vector.tensor_scalar(txc2[:], txc[:], 0.0, None, _alu("max"))
                idxf = tmp("p6", KF)
                nc.vector.tensor_add(idxf[:], tyc2[:], txc2[:])
                idxf1 = tmp("p7", KF)
                nc.vector.tensor_scalar(idxf1[:], idxf[:], 1.0, float(NE - 1),
                                        _alu("add"), _alu("min"))
                idxc = tmp("ic0", KF, I16)
                nc.vector.tensor_copy(idxc[:], idxf[:])
                idxc1 = tmp("ic1", KF, I16)
                nc.vector.tensor_copy(idxc1[:], idxf1[:])

                # replicate -> wrapped per-image idx buffers via DRAM
                # idxw[im] [128, 4*KF], col = sub*KF + (k*64 + sr)
                nc.sync.dma_start(istage[0].ap(), idxc[:])
                nc.sync.dma_start(istage[1].ap(), idxc1[:])
                for im in range(2):
                    for half in range(2):
                        ssrc = istage[half].ap()[im * 64:(im + 1) * 64, :] \
                            .rearrange("(s p) c -> p s c", s=4)
                        for gm in range(4):
                            lo = half * 64 + gm * 16
                            dst = idxws[im][lo:lo + 16, :].rearrange(
                                "p (s c) -> p s c", s=4)
                            nc.sync.dma_start(dst, ssrc)

                # --- weight pipeline (compact [128, WF]) ---
                byt = ld("byt", WF, "byt")
                dyt = ld("dyt", WF, "dyt")
                pyt = tmp("p1")
                nc.vector.tensor_add(pyt[:], byt[:], dyt[:])
                y0t = tmp("v1")
                floor_(pyt, y0t, WF)
                fy = tmp("p2")
                nc.vector.tensor_sub(fy[:], pyt[:], y0t[:])
                Y0 = tmp("Y0")
                nc.vector.tensor_scalar(Y0[:], fy[:], -1.0, 1.0,
                                        _alu("mult"), _alu("add"))
                ta = tmp("p3")
                nc.vector.tensor_scalar(ta[:], y0t[:], 0.0, None, _alu("is_ge"))
                tb = tmp("p4")
                nc.vector.tensor_scalar(tb[:], y0t[:], 65.0, None, _alu("is_le"))
                nc.vector.tensor_mul(ta[:], ta[:], tb[:])  # vy0
                nc.vector.tensor_mul(Y0[:], Y0[:], ta[:])
                nc.vector.tensor_scalar(ta[:], y0t[:], -1.0, None, _alu("is_ge"))
                nc.vector.tensor_scalar(tb[:], y0t[:], 64.0, None, _alu("is_le"))
                nc.vector.tensor_mul(ta[:], ta[:], tb[:])  # vy1
                Y1 = tmp("Y1")
                nc.vector.tensor_mul(Y1[:], fy[:], ta[:])

                bxt = ld("bxt", WF, "byt")
                dxt = ld("dxt", WF, "dyt")
                pxt = tmp("p1")
                nc.vector.tensor_add(pxt[:], bxt[:], dxt[:])
                x0t = tmp("v2")
                floor_(pxt, x0t, WF)
                fx = tmp("p2")
                nc.vector.tensor_sub(fx[:], pxt[:], x0t[:])
                X0 = tmp("X0")
                nc.vector.tensor_scalar(X0[:], fx[:], -1.0, 1.0,
                                        _alu("mult"), _alu("add"))
                nc.vector.tensor_scalar(ta[:], x0t[:], 0.0, None, _alu("is_ge"))
                nc.vector.tensor_scalar(tb[:], x0t[:], 65.0, None, _alu("is_le"))
                nc.vector.tensor_mul(ta[:], ta[:], tb[:])  # vx0
                nc.vector.tensor_mul(X0[:], X0[:], ta[:])
                nc.vector.tensor_scalar(ta[:], x0t[:], -1.0, None, _alu("is_ge"))
                nc.vector.tensor_scalar(tb[:], x0t[:], 64.0, None, _alu("is_le"))
                nc.vector.tensor_mul(ta[:], ta[:], tb[:])  # vx1
                X1 = tmp("X1")
                nc.vector.tensor_mul(X1[:], fx[:], ta[:])
                # X = X0 + hsel*(X1-X0)
                nc.vector.tensor_sub(X1[:], X1[:], X0[:])
                nc.vector.tensor_scalar(X1[:], X1[:], hsel[:], None,
                                        _alu("mult"))
                nc.vector.tensor_add(X0[:], X0[:], X1[:])

                # wc [128, 2*WF] f32r, col = (img*KF + k*64 + i)*2 + j
                wc = tp.tile([128, 2 * WF], F32R, tag="wc", name="wc")
                wcv = wc[:].rearrange("v (c j) -> v c j", j=2)
                mw0 = nc.vector.tensor_mul(wcv[:, :, 0], Y0[:], X0[:])
                mw1 = nc.vector.tensor_mul(wcv[:, :, 1], Y1[:], X0[:])

                # reshuffle via DRAM: [128=(t256 4? ...)] see host docstring
                nc.sync.dma_start(stage_d.ap(), wc[:])
                stg5 = stage_d.ap().rearrange(
                    "(t b h) (mk ij) -> t b h mk ij",
                    t=16, b=4, h=2, mk=2 * K, ij=128)
                for h in range(2):
                    dsth = wc32[h * 16:(h + 1) * 16, :].rearrange(
                        "p (mk b ij) -> p mk b ij", mk=2 * K, b=4)
                    for b in range(4):
                        nc.sync.dma_start(dsth[:, :, b, :],
                                          stg5[:, b, h, :, :])

            # ---------- phase 2: patch tables ----------
            tabs = []
            with tc.tile_pool(name="imgs", bufs=1) as ipool:
                for nm in ("imga", "imgb"):
                    img = ipool.tile([128, HIN * WIN], F32, tag=nm, name=nm + "_t")
                    nc.sync.dma_start(img[:], din[nm].ap())
                    tab = cpool.tile([128, NE * 2], VAL_DT, name=nm + "_tab")
                    nc.gpsimd.memset(tab[:, 0:TE * 2], 0.0)
                    nc.gpsimd.memset(tab[:, 66 * TE * 2:68 * TE * 2], 0.0)
                    t3 = tab[:].rearrange("p (t e) -> p t e", e=TE * 2)
                    nc.gpsimd.memset(t3[:, :, 0:2], 0.0)
                    nc.gpsimd.memset(t3[:, :, 134:136], 0.0)
                    tabv = tab[:].rearrange("p (n d) -> p n d", d=2)
                    imgv = img[:].rearrange("p (h w) -> p h w", w=WIN)
                    for jy in (0, 1):
                        ty0 = 1 - jy
                        dst = tabv[:, :, jy].rearrange(
                            "p (ty tx) -> p ty tx", tx=TE)
                        nc.scalar.copy(dst[:, ty0:ty0 + HIN, 1:1 + WIN],
                                       imgv[:])
                    tabs.append(tabv)

            # ---------- phase 3: main loop ----------
            with tc.tile_pool(name="gat", bufs=10) as gpool, \
                 tc.tile_pool(name="smul", bufs=6) as spool, \
                 tc.tile_pool(name="outs", bufs=4) as opool, \
                 tc.tile_pool(name="wps", bufs=2, space="PSUM") as wps, \
                 tc.tile_pool(name="ops", bufs=2, space="PSUM") as ops_:
                wcb = wc32[:].rearrange("v (g q) -> v g q", q=512)
                for t in range(NCHUNK):
                    out_ps2 = [
                        ops_.tile([64, CHUNK], F32, tag=f"ops{i}",
                                  name=f"ops{i}_{t}") for i in range(2)]
                    for im in range(2):
                        idxwv = idxws[im][:].rearrange(
                            "q (s c) -> q s c", s=4)
                        sub, soff = t // 2, (t % 2) * 32
                        for k in range(K):
                            g = gpool.tile([128, CHUNK, 2], VAL_DT, tag="g", name=f"g_{t}_{im}_{k}")
                            nc.gpsimd.ap_gather(
                                g[:], tabs[im],
                                idxwv[:, sub,
                                      k * 64 + soff:k * 64 + soff + 32],
                                channels=128, num_elems=NE, d=2,
                                num_idxs=CHUNK)
                            wp = wps.tile([128, CHUNK * 2], F32, tag="wp", name=f"wp_{t}_{im}_{k}")
                            for q in range(2):
                                tt = t * 2 + q
                                nc.tensor.matmul(
                                    wp[:, q * 512:(q + 1) * 512],
                                    ind2[:, tt * 128:(tt + 1) * 128],
                                    wcb[:, im * K + k, :],
                                    start=True, stop=True)
                            s = spool.tile([128, CHUNK * 2], F32R, tag="s", name=f"s_{t}_{im}_{k}")
                            nc.vector.tensor_tensor(
                                s[:], g[:].rearrange("p n d -> p (n d)"),
                                wp[:], _alu("mult"))
                            sv = s[:].rearrange("p (n d) -> p n d", d=2)
                            for j in range(2):
                                nc.tensor.matmul(
                                    out_ps2[im][:, :],
                                    wmat[:, k * 64:(k + 1) * 64],
                                    sv[:, :, j],
                                    start=(k == 0 and j == 0),
                                    stop=(k == 8 and j == 1))
                    for im in range(2):
                        ot = opool.tile([64, CHUNK], F32, tag=f"ot{im}",
                                        name=f"ot{im}_{t}")
                        nc.scalar.copy(ot[:], out_ps2[im][:, :])
                        nc.sync.dma_start(
                            out_d.ap()[im * 64:(im + 1) * 64,
                                       t * CHUNK:(t + 1) * CHUNK], ot[:])

    nc.compile()
    return nc


# ---------------- host side ----------------

def _host_arrays(input, offset, weight):
    inp = np.ascontiguousarray(input, dtype=np.float32)
    off = np.ascontiguousarray(offset, dtype=np.float32)
    w = np.ascontiguousarray(weight, dtype=np.float32)

    wk = w.reshape(COUT, CIN, K)
    wcko = wk.transpose(1, 2, 0)  # [c, k, o]
    wmat = np.empty((128, K * 64), np.float32)
    wmat[0:64] = wcko.reshape(64, K * 64)
    wmat[64:128] = wcko.reshape(64, K * 64)

    ind2 = np.zeros((32, 16, 128), np.float32)
    for tt in range(16):
        ind2[tt, tt, 0:64] = 1.0
        ind2[16 + tt, tt, 64:128] = 1.0
    ind2 = ind2.reshape(32, 2048)
    hsel = (np.arange(128) % 2).astype(np.float32).reshape(128, 1)

    P = np.arange(NPX)
    ho = (P // WO).astype(np.float32)
    wo = (P % WO).astype(np.float32)
    kh = (np.arange(K) // KW).astype(np.float32)
    kw = (np.arange(K) % KW).astype(np.float32)

    # idx-compact: u = img*64 + sub*16 + pp, col = k*64 + sr,
    # pixel = (sub*64+sr)*16 + pp
    u = np.arange(128)
    img_u = u // 64
    sub_u = (u % 64) // 16
    pp_u = u % 16
    sr = np.arange(64)
    pix_c = (sub_u[:, None] * 64 + sr[None, :]) * 16 + pp_u[:, None]

    # weight-compact: v = 2*t64 + h, col = img*KF + k*64 + i, P = t64*64 + i
    v = np.arange(128)
    t64_v = v // 2
    i_col = np.arange(64)
    pix_t = t64_v[:, None] * 64 + i_col[None, :]

    def expand(base_vals, tap_off, pix):
        b = base_vals[pix]
        return np.ascontiguousarray(
            (b[:, None, :] + tap_off[None, :, None]).reshape(128, -1))

    byc = expand(ho, kh, pix_c)
    bxc = expand(wo, kw, pix_c)
    byt1 = expand(ho, kh, pix_t)
    bxt1 = expand(wo, kw, pix_t)
    byt = np.concatenate([byt1, byt1], axis=1)
    bxt = np.concatenate([bxt1, bxt1], axis=1)

    offr = off.reshape(N, K, 2, NPX)

    in_maps = []
    for core in range(NCORES):
        na, nb = 2 * core, 2 * core + 1

        def img128(n):
            a = np.empty((128, HIN * WIN), np.float32)
            a[0:64] = inp[n].reshape(64, -1)
            a[64:128] = inp[n].reshape(64, -1)
            return a

        # dyc/dxc [128, KF]
        dy_ab = offr[[na, nb]][:, :, 0, :]  # [2, K, NPX]
        dx_ab = offr[[na, nb]][:, :, 1, :]
        dyc = dy_ab[img_u[:, None], :, pix_c].reshape(128, KF)
        dxc = dx_ab[img_u[:, None], :, pix_c].reshape(128, KF)
        # fancy-index note: dy_ab[img, :, pix] with img [128,1], pix [128,64]
        # gives [128, 64, K]; need [128, K, 64] -> transpose
        dyc = dy_ab[img_u[:, None], :, pix_c].transpose(0, 2, 1).reshape(128, KF)
        dxc = dx_ab[img_u[:, None], :, pix_c].transpose(0, 2, 1).reshape(128, KF)

        # dyt/dxt [128, WF] col = img*KF + k*64 + i
        def wt(arr):
            # arr [2, K, NPX] -> [128, 2, K, 64] -> [128, WF]
            g = arr[:, :, pix_t]  # [2, K, 128, 64]
            return np.ascontiguousarray(
                g.transpose(2, 0, 1, 3).reshape(128, WF))

        dyt = wt(dy_ab)
        dxt = wt(dx_ab)

        in_maps.append(dict(
            imga=img128(na), imgb=img128(nb), wmat=wmat, ind2=ind2,
            hsel=hsel, dyc=dyc, dxc=dxc, byc=byc, bxc=bxc,
            dyt=dyt, dxt=dxt, byt=byt, bxt=bxt,
        ))
    return in_maps


_NC_CACHE = None


def get_nc():
    global _NC_CACHE
    if _NC_CACHE is None:
        _NC_CACHE = build_bass()
    return _NC_CACHE


def kernel(input, offset, weight, _trace=False):
    from concourse.bass_utils import run_bass_kernel_spmd

    nc = get_nc()
    in_maps = _host_arrays(np.asarray(input), np.asarray(offset),
                           np.asarray(weight))
    res = run_bass_kernel_spmd(nc, in_maps, list(range(NCORES)), trace=_trace)
    out = np.empty((N, COUT, HO, WO), np.float32)
    for core in range(NCORES):
        o128 = np.asarray(res.results[core]["out128"])
        out[2 * core] = o128[0:64].reshape(COUT, HO, WO)
        out[2 * core + 1] = o128[64:128].reshape(COUT, HO, WO)
    if _trace:
        return out, res
    return out



# revision 4
# speedup vs baseline: 2.1378x; 2.1378x over previous
"""Deformable conv2d (torchvision semantics: stride=1, pad=0, dil=1,
offset_groups=1, no mask/bias) on 8 TRN2 NeuronCores via Bass/Tile.

Hardcoded shapes: input [16,64,66,66] f32, offset [16,18,64,64] f32,
weight [64,64,3,3] f32 -> out [16,64,64,64] f32.

Sharding: data-parallel over batch; core i handles images (2i, 2i+1).

Per-core scheme (partitions p = (img in {0,1}, channel c)):
  - patch table tab[p, ty*68+tx, j] = I[c(p), ty-1+jy, tx-1+jx] (j=2*jx+jy):
    each entry holds the 2x2 bilinear patch anchored at (ty-1, tx-1), so ONE
    gpsimd ap_gather index (d=4) fetches all four corners for a (pixel, tap)
    sample. OOB entries are zeroed and always weight-masked.
  - per-corner bilinear weights (validity folded in) are computed compactly,
    then broadcast across channel partitions with an indicator matmul (PSUM).
  - DVE: S = G * W, then pairwise j-reduction; TensorE: per tap one matmul
    per image (lhsT = per-image zero-padded weights), accumulating 9 taps in
    PSUM.
"""

import sys

sys.path.insert(0, "/opt/trn_rl_repo")

import numpy as np

import concourse.bacc as bacc
import concourse.mybir as mybir
import concourse.tile as tile

F32 = mybir.dt.float32
BF16 = mybir.dt.bfloat16
I16 = mybir.dt.int16
I32 = mybir.dt.int32

N, CIN, COUT = 16, 64, 64
HIN, WIN = 66, 66
KH, KW = 3, 3
HO, WO = 64, 64
K = KH * KW
NPX = HO * WO
NCORES = 8

TE = 68
NE = TE * TE  # 4624
CHUNK = 512
NCHUNK = NPX // CHUNK  # 8
KF = K * 64  # 576


def _alu(name):
    return getattr(mybir.AluOpType, name)


def build_bass():
    nc = bacc.Bacc("TRN2", target_bir_lowering=False, debug=False,
                   num_devices=NCORES)

    din = {}
    for nm, shp in [
        ("img", [128, HIN * WIN]),
        ("wmats", [128, 2 * KF]), ("ind", [32, 16 * 128]),
        ("dyc", [128, KF]), ("dxc", [128, KF]),
        ("byc", [128, KF]), ("bxc", [128, KF]),
        ("dyt", [128, KF]), ("dxt", [128, KF]),
        ("byt", [128, KF]), ("bxt", [128, KF]),
    ]:
        din[nm] = nc.dram_tensor(nm, shp, F32, kind="ExternalInput")
    out_d = nc.dram_tensor("out128", [128, NPX], F32, kind="ExternalOutput")
    istage = nc.dram_tensor("idxstage", [128, KF], I16)
    wstage = nc.dram_tensor("wstage", [128, 4 * KF], BF16)

    with tile.TileContext(nc) as tc:
        with tc.tile_pool(name="cst", bufs=1) as cpool:
            # persistent tiles
            tab = cpool.tile([128, NE * 4], BF16, name="tab")
            idxw = cpool.tile([128, 4 * KF], I16, name="idxw")
            wc32 = cpool.tile([32, 16 * KF], BF16, name="wc32")
            ind = cpool.tile([32, 16 * 128], BF16, name="ind")
            wm = cpool.tile([128, 2 * KF], BF16, name="wm")

            # zero the table border entries that the shifted copies below
            # don't cover (their corners are always OOB -> weight-masked,
            # but gathered values must be finite)
            t4 = tab[:].rearrange("p (ty tx j) -> p ty tx j", tx=TE, j=4)
            nc.vector.memset(t4[:, 0:1, :, :], 0.0)
            nc.vector.memset(t4[:, 66:68, :, :], 0.0)
            nc.vector.memset(t4[:, :, 0:1, :], 0.0)
            nc.vector.memset(t4[:, :, 66:68, :], 0.0)

            # ---------- phase 1: idx + weight pipelines, patch table ------
            with tc.tile_pool(name="pipe", bufs=1) as tp:
                img = tp.tile([128, HIN * WIN], F32, tag="img", name="img_t")
                nc.sync.dma_start(img[:], din["img"].ap())

                _cnt = [0]

                def _nm(tg):
                    _cnt[0] += 1
                    return f"{tg}_{_cnt[0]}"

                def ld(nm, tg):
                    t = tp.tile([128, KF], F32, tag=tg, name=_nm(nm))
                    nc.sync.dma_start(t[:], din[nm].ap())
                    return t

                def tmp(tg, dt=F32):
                    return tp.tile([128, KF], dt, tag=tg, name=_nm(tg))

                def floor_(x, out):
                    ti = tmp("fl_i", I32)
                    nc.vector.tensor_copy(ti[:], x[:])
                    tf = tmp("fl_f")
                    nc.vector.tensor_copy(tf[:], ti[:])
                    co = tmp("fl_c")
                    nc.vector.tensor_tensor(co[:], tf[:], x[:], _alu("is_gt"))
                    nc.vector.tensor_sub(out[:], tf[:], co[:])

                # --- idx pipeline (wrapped compact [128, KF]) ---
                byc = ld("byc", "byc")
                dyc = ld("dyc", "dyc")
                pyc = tmp("p1")
                nc.vector.tensor_add(pyc[:], byc[:], dyc[:])
                y0c = tmp("v1")
                floor_(pyc, y0c)
                tyc = tmp("p2")
                nc.vector.tensor_scalar(tyc[:], y0c[:], 1.0, 67.0,
                                        _alu("add"), _alu("min"))
                tyc2 = tmp("p3")
                nc.vector.tensor_scalar(tyc2[:], tyc[:], 0.0, float(TE),
                                        _alu("max"), _alu("mult"))
                bxc = ld("bxc", "byc")
                dxc = ld("dxc", "dyc")
                pxc = tmp("p1")
                nc.vector.tensor_add(pxc[:], bxc[:], dxc[:])
                x0c = tmp("v2")
                floor_(pxc, x0c)
                txc = tmp("p4")
                nc.vector.tensor_scalar(txc[:], x0c[:], 1.0, 67.0,
                                        _alu("add"), _alu("min"))
                txc2 = tmp("p5")
                nc.vector.tensor_scalar(txc2[:], txc[:], 0.0, None,
                                        _alu("max"))
                idxf = tmp("p6")
                nc.vector.tensor_add(idxf[:], tyc2[:], txc2[:])
                idxc = tmp("ic0", I16)
                nc.vector.tensor_copy(idxc[:], idxf[:])

                # replicate wrapped idx to the 4 channel groups per image
                nc.sync.dma_start(istage.ap(), idxc[:])
                for im in range(2):
                    ssrc = istage.ap()[im * 64:(im + 1) * 64, :] \
                        .rearrange("(s p) c -> p s c", s=4)
                    for gm in range(4):
                        lo = im * 64 + gm * 16
                        dst = idxw[lo:lo + 16, :].rearrange(
                            "p (s c) -> p s c", s=4)
                        nc.sync.dma_start(dst, ssrc)

                # --- patch table (4 shifted copies, f32 -> bf16) ---
                imgv = img[:].rearrange("p (h w) -> p h w", w=WIN)
                for jy in (0, 1):
                    for jx in (0, 1):
                        j = 2 * jx + jy
                        dst = t4[:, 1 - jy:67 - jy, 1 - jx:67 - jx, j]
                        nc.scalar.copy(dst, imgv[:])

                # --- weight pipeline (raster compact [128, KF]) ---
                byt = ld("byt", "byt")
                dyt = ld("dyt", "dyt")
                pyt = tmp("p1")
                nc.vector.tensor_add(pyt[:], byt[:], dyt[:])
                y0t = tmp("v1")
                floor_(pyt, y0t)
                fy = tmp("p2")
                nc.vector.tensor_sub(fy[:], pyt[:], y0t[:])
                Y0 = tmp("Y0")
                nc.vector.tensor_scalar(Y0[:], fy[:], -1.0, 1.0,
                                        _alu("mult"), _alu("add"))
                ta = tmp("p3")
                nc.vector.tensor_scalar(ta[:], y0t[:], 0.0, None,
                                        _alu("is_ge"))
                tb = tmp("p4")
                nc.vector.tensor_scalar(tb[:], y0t[:], 65.0, None,
                                        _alu("is_le"))
                nc.vector.tensor_mul(ta[:], ta[:], tb[:])  # vy0
                nc.vector.tensor_mul(Y0[:], Y0[:], ta[:])
                nc.vector.tensor_scalar(ta[:], y0t[:], -1.0, None,
                                        _alu("is_ge"))
                nc.vector.tensor_scalar(tb[:], y0t[:], 64.0, None,
                                        _alu("is_le"))
                nc.vector.tensor_mul(ta[:], ta[:], tb[:])  # vy1
                Y1 = tmp("Y1")
                nc.vector.tensor_mul(Y1[:], fy[:], ta[:])

                bxt = ld("bxt", "byt")
                dxt = ld("dxt", "dyt")
                pxt = tmp("p1")
                nc.vector.tensor_add(pxt[:], bxt[:], dxt[:])
                x0t = tmp("v2")
                floor_(pxt, x0t)
                fx = tmp("p2")
                nc.vector.tensor_sub(fx[:], pxt[:], x0t[:])
                X0 = tmp("X0")
                nc.vector.tensor_scalar(X0[:], fx[:], -1.0, 1.0,
                                        _alu("mult"), _alu("add"))
                nc.vector.tensor_scalar(ta[:], x0t[:], 0.0, None,
                                        _alu("is_ge"))
                nc.vector.tensor_scalar(tb[:], x0t[:], 65.0, None,
                                        _alu("is_le"))
                nc.vector.tensor_mul(ta[:], ta[:], tb[:])  # vx0
                nc.vector.tensor_mul(X0[:], X0[:], ta[:])
                nc.vector.tensor_scalar(ta[:], x0t[:], -1.0, None,
                                        _alu("is_ge"))
                nc.vector.tensor_scalar(tb[:], x0t[:], 64.0, None,
                                        _alu("is_le"))
                nc.vector.tensor_mul(ta[:], ta[:], tb[:])  # vx1
                X1 = tmp("X1")
                nc.vector.tensor_mul(X1[:], fx[:], ta[:])

                # products -> wcc [128, (k s) j] bf16, j = 2*jx + jy
                wcc = tp.tile([128, 4 * KF], BF16, tag="wcc", name="wcc")
                wccv = wcc[:].rearrange("p (c j) -> p c j", j=4)
                nc.vector.tensor_mul(wccv[:, :, 0], Y0[:], X0[:])
                nc.vector.tensor_mul(wccv[:, :, 1], Y1[:], X0[:])
                nc.vector.tensor_mul(wccv[:, :, 2], Y0[:], X1[:])
                nc.vector.tensor_mul(wccv[:, :, 3], Y1[:], X1[:])

                # reshuffle via DRAM into wc32 [32=(im,th), k*1024+(q*64+s)*4+j]
                nc.sync.dma_start(wstage.ap(), wcc[:])
                src4 = wstage.ap().rearrange(
                    "(imth q) (k sj) -> imth q k sj", q=4, k=K)
                dst4 = wc32[:].rearrange(
                    "v (k q sj) -> v k q sj", k=K, q=4)
                for q in range(4):
                    nc.sync.dma_start(dst4[:, :, q, :], src4[:, q, :, :])

                # selector + conv weights, cast to bf16
                indf = tp.tile([32, 16 * 128], F32, tag="indf", name="indf")
                nc.sync.dma_start(indf[:], din["ind"].ap())
                nc.vector.tensor_copy(ind[:], indf[:])
                wmf = tp.tile([128, 2 * KF], F32, tag="wmf", name="wmf")
                nc.sync.dma_start(wmf[:], din["wmats"].ap())
                nc.vector.tensor_copy(wm[:], wmf[:])

            # ---------- phase 2: main loop ----------
            tabv = tab[:].rearrange("p (n d) -> p n d", d=4)
            idxwv = idxw[:].rearrange("q (s c) -> q s c", s=4)
            with tc.tile_pool(name="gat", bufs=8) as gpool, \
                 tc.tile_pool(name="smul", bufs=4) as spool, \
                 tc.tile_pool(name="red1", bufs=4) as rpool, \
                 tc.tile_pool(name="red2", bufs=6) as r2pool, \
                 tc.tile_pool(name="outs", bufs=4) as opool, \
                 tc.tile_pool(name="wps", bufs=4, space="PSUM") as wps, \
                 tc.tile_pool(name="ops", bufs=2, space="PSUM") as ops_:
                for t in range(NCHUNK):
                    sub, soff = t // 2, (t % 2) * 32
                    out_ps2 = [
                        ops_.tile([64, CHUNK], F32, tag=f"ops{i}",
                                  name=f"ops{i}_{t}") for i in range(2)]
                    for k in range(K):
                        g = gpool.tile([128, CHUNK, 4], BF16, tag="g",
                                       name=f"g_{t}_{k}")
                        nc.gpsimd.ap_gather(
                            g[:], tabv,
                            idxwv[:, sub, k * 64 + soff:k * 64 + soff + 32],
                            channels=128, num_elems=NE, d=4, num_idxs=CHUNK)
                        s = spool.tile([128, CHUNK * 4], BF16, tag="s",
                                       name=f"s_{t}_{k}")
                        gf = g[:].rearrange("p n d -> p (n d)")
                        for q in range(4):
                            th = t * 2 + q // 2
                            wp = wps.tile([128, 512], F32, tag="wp",
                                          name=f"wp_{t}_{k}_{q}")
                            nc.tensor.matmul(
                                wp[:],
                                ind[:, th * 128:(th + 1) * 128],
                                wc32[:, k * 1024 + (q % 2) * 512:
                                     k * 1024 + (q % 2) * 512 + 512],
                                start=True, stop=True)
                            nc.vector.tensor_tensor(
                                s[:, q * 512:(q + 1) * 512],
                                gf[:, q * 512:(q + 1) * 512],
                                wp[:], _alu("mult"))
                        sv = s[:].rearrange("p (n j) -> p n j", j=4)
                        r1 = rpool.tile([128, CHUNK, 2], BF16, tag="r1",
                                        name=f"r1_{t}_{k}")
                        nc.vector.tensor_add(r1[:], sv[:, :, 0:2],
                                             sv[:, :, 2:4])
                        r2 = r2pool.tile([128, CHUNK], BF16, tag="r2",
                                         name=f"r2_{t}_{k}")
                        nc.vector.tensor_add(r2[:], r1[:, :, 0], r1[:, :, 1])
                        for im in range(2):
                            nc.tensor.matmul(
                                out_ps2[im][:, :],
                                wm[:, im * KF + k * 64:im * KF + k * 64 + 64],
                                r2[:],
                                start=(k == 0), stop=(k == 8))
                    for im in range(2):
                        ot = opool.tile([64, CHUNK], F32, tag=f"ot{im}",
                                        name=f"ot{im}_{t}")
                        nc.scalar.copy(ot[:], out_ps2[im][:, :])
                        nc.sync.dma_start(
                            out_d.ap()[im * 64:(im + 1) * 64,
                                       t * CHUNK:(t + 1) * CHUNK], ot[:])

    nc.compile()
    return nc


# ---------------- host side ----------------

def _host_arrays(input, offset, weight):
    inp = np.ascontiguousarray(input, dtype=np.float32)
    off = np.ascontiguousarray(offset, dtype=np.float32)
    w = np.ascontiguousarray(weight, dtype=np.float32)

    wk = w.reshape(COUT, CIN, K)
    block = np.ascontiguousarray(
        wk.transpose(1, 2, 0).reshape(64, KF))  # [c, k*64+o]
    wmats = np.zeros((128, 2 * KF), np.float32)
    wmats[0:64, 0:KF] = block
    wmats[64:128, KF:2 * KF] = block

    ind = np.zeros((32, 16, 128), np.float32)
    for v in range(32):
        imv, thv = v // 16, v % 16
        ind[v, thv, imv * 64:(imv + 1) * 64] = 1.0
    ind = ind.reshape(32, 16 * 128)

    P = np.arange(NPX)
    ho = (P // WO).astype(np.float32)
    wo = (P % WO).astype(np.float32)
    kh = (np.arange(K) // KW).astype(np.float32)
    kw = (np.arange(K) % KW).astype(np.float32)

    u = np.arange(128)
    im_u = u // 64
    # wrapped (idx pipeline): pixel = (sub*64+sr)*16 + pp
    sub_u = (u % 64) // 16
    pp_u = u % 16
    sr = np.arange(64)
    pix_c = (sub_u[:, None] * 64 + sr[None, :]) * 16 + pp_u[:, None]
    # raster (weight pipeline): pixel = (u%64)*64 + s
    pix_t = (u % 64)[:, None] * 64 + sr[None, :]

    def expand(base_vals, tap_off, pix):
        b = base_vals[pix]  # [128, 64]
        return np.ascontiguousarray(
            (b[:, None, :] + tap_off[None, :, None]).reshape(128, KF))

    byc = expand(ho, kh, pix_c)
    bxc = expand(wo, kw, pix_c)
    byt = expand(ho, kh, pix_t)
    bxt = expand(wo, kw, pix_t)

    offr = off.reshape(N, K, 2, NPX)

    in_maps = []
    for core in range(NCORES):
        na, nb = 2 * core, 2 * core + 1
        img = np.empty((128, HIN * WIN), np.float32)
        img[0:64] = inp[na].reshape(64, -1)
        img[64:128] = inp[nb].reshape(64, -1)

        dy_ab = offr[[na, nb]][:, :, 0, :]  # [2, K, NPX]
        dx_ab = offr[[na, nb]][:, :, 1, :]

        def relay(arr, pix):
            g = arr[im_u[:, None], :, pix]  # [128, 64, K]
            return np.ascontiguousarray(
                g.transpose(0, 2, 1).reshape(128, KF))

        in_maps.append(dict(
            img=img, wmats=wmats, ind=ind,
            dyc=relay(dy_ab, pix_c), dxc=relay(dx_ab, pix_c),
            byc=byc, bxc=bxc,
            dyt=relay(dy_ab, pix_t), dxt=relay(dx_ab, pix_t),
            byt=byt, bxt=bxt,
        ))
    return in_maps


_NC_CACHE = None


def get_nc():
    global _NC_CACHE
    if _NC_CACHE is None:
        _NC_CACHE = build_bass()
    return _NC_CACHE


def kernel(input, offset, weight, _trace=False):
    from concourse.bass_utils import run_bass_kernel_spmd

    nc = get_nc()
    in_maps = _host_arrays(np.asarray(input), np.asarray(offset),
                           np.asarray(weight))
    res = run_bass_kernel_spmd(nc, in_maps, list(range(NCORES)), trace=_trace)
    out = np.empty((N, COUT, HO, WO), np.float32)
    for core in range(NCORES):
        o128 = np.asarray(res.results[core]["out128"])
        out[2 * core] = o128[0:64].reshape(COUT, HO, WO)
        out[2 * core + 1] = o128[64:128].reshape(COUT, HO, WO)
    if _trace:
        return out, res
    return out


# revision 16
# speedup vs baseline: 3.5923x; 1.6804x over previous
"""Deformable conv2d (torchvision semantics: stride=1, pad=0, dil=1,
offset_groups=1, no mask/bias) on 8 TRN2 NeuronCores via Bass/Tile.

Hardcoded shapes: input [16,64,66,66] f32, offset [16,18,64,64] f32,
weight [64,64,3,3] f32 -> out [16,64,64,64] f32.

Sharding: data-parallel over batch; core i handles images (2i, 2i+1).

Per-core scheme (partitions p = (img, half, cpair)):
  - patch table tab[p, (ty*68+tx)*8 + j*2 + cc] = I[img][2*cpair+cc,
    ty-1+jy, tx-1+jx] (j=2*jx+jy): one gpsimd ap_gather index (d=8)
    fetches the 2x2 bilinear corners of TWO channels at once. Each image's
    pixels are split into two half-streams (partition halves carry
    separate index streams), halving every Q7 core's serial index load —
    the gather's per-RD-command latency is the kernel's bottleneck.
  - per-corner bilinear weights (validity folded in) live compactly in
    wc64 rows keyed by 128-pixel blocks; a [64,128] indicator matmul
    broadcasts them across channel partitions into PSUM (wp), cc-expanded
    for free via a stride-0 DVE view.
  - DVE: S = G * W, then pairwise j-reduction; TensorE: per (tap, img,
    half, cc) one matmul with zero-padded lhsT accumulating into the
    (img, half) PSUM output region over all 18 (tap, cc) steps.
"""

import sys

sys.path.insert(0, "/opt/trn_rl_repo")

import numpy as np

import concourse.bacc as bacc
import concourse.mybir as mybir
import concourse.tile as tile

F32 = mybir.dt.float32
BF16 = mybir.dt.bfloat16
I16 = mybir.dt.int16
I32 = mybir.dt.int32

N, CIN, COUT = 16, 64, 64
HIN, WIN = 66, 66
KH, KW = 3, 3
HO, WO = 64, 64
K = KH * KW
NPX = HO * WO
NCORES = 8

TE = 68
NE = TE * TE  # 4624
CHUNK = 512
NCHUNK = NPX // CHUNK  # 8
KF = K * 64  # 576


def _alu(name):
    return getattr(mybir.AluOpType, name)


def build_bass():
    nc = bacc.Bacc("TRN2", target_bir_lowering=False, debug=False,
                   num_devices=NCORES)

    din = {}
    for nm, shp in [
        ("img2a", [128, 33 * WIN * 2]), ("img2b", [128, 33 * WIN * 2]),
        ("wm8", [128, 8 * KF]), ("ind64", [64, 16 * 128]),
        ("dyc", [128, KF]), ("dxc", [128, KF]),
        ("byc", [128, KF]), ("bxc", [128, KF]),
        ("dyt", [128, KF]), ("dxt", [128, KF]),
        ("byt", [128, KF]), ("bxt", [128, KF]),
    ]:
        din[nm] = nc.dram_tensor(nm, shp, F32, kind="ExternalInput")
    out_d = nc.dram_tensor("out128", [128, NPX], F32, kind="ExternalOutput")
    istage = nc.dram_tensor("idxstage", [128, KF], I16)
    wstage = nc.dram_tensor("wstage", [128, 4 * KF], BF16)

    with tile.TileContext(nc) as tc:
        with tc.tile_pool(name="cst", bufs=1) as cpool:
            # persistent tiles
            tab = cpool.tile([128, NE * 8], BF16, name="tab")
            idxw = cpool.tile([128, 2 * KF], I16, name="idxw")
            wc64 = cpool.tile([64, K * 512], BF16, name="wc64")
            ind64 = cpool.tile([64, 16 * 128], BF16, name="ind64")
            wm8 = cpool.tile([128, 8 * KF], BF16, name="wm8")
            gbufs = [cpool.tile([128, 256, 8], BF16, name=f"gbuf{i}")
                     for i in range(5)]

            # zero the table border entries the shifted copies don't cover
            t8 = tab[:].rearrange("p (ty tx d) -> p ty tx d", tx=TE, d=8)
            nc.gpsimd.memset(t8[:, 0:1, :, :], 0.0)
            nc.gpsimd.memset(t8[:, 66:68, :, :], 0.0)
            nc.gpsimd.memset(t8[:, :, 0:1, :], 0.0)
            nc.gpsimd.memset(t8[:, :, 66:68, :], 0.0)

            # ---------- phase 1: idx + weight pipelines, patch table ------
            with tc.tile_pool(name="pipe", bufs=1) as tp:
                _cnt = [0]

                def _nm(tg):
                    _cnt[0] += 1
                    return f"{tg}_{_cnt[0]}"

                def ld(nm, tg):
                    t = tp.tile([128, KF], F32, tag=tg, name=_nm(nm))
                    nc.sync.dma_start(t[:], din[nm].ap())
                    return t

                def tmp(tg, dt=F32):
                    return tp.tile([128, KF], dt, tag=tg, name=_nm(tg))

                def floor_(x, out):
                    ti = tmp("fl_i", I32)
                    nc.vector.tensor_copy(ti[:], x[:])
                    tf = tmp("fl_f")
                    nc.vector.tensor_copy(tf[:], ti[:])
                    co = tmp("fl_c")
                    nc.vector.tensor_tensor(co[:], tf[:], x[:], _alu("is_gt"))
                    nc.vector.tensor_sub(out[:], tf[:], co[:])

                # --- idx pipeline (stream-wrapped compact [128, KF]) ---
                byc = ld("byc", "byc")
                dyc = ld("dyc", "dyc")
                pyc = tmp("p1")
                nc.vector.tensor_add(pyc[:], byc[:], dyc[:])
                y0c = tmp("v1")
                floor_(pyc, y0c)
                tyc = tmp("p2")
                nc.vector.tensor_scalar(tyc[:], y0c[:], 1.0, 67.0,
                                        _alu("add"), _alu("min"))
                tyc2 = tmp("p3")
                nc.vector.tensor_scalar(tyc2[:], tyc[:], 0.0, float(TE),
                                        _alu("max"), _alu("mult"))
                bxc = ld("bxc", "byc")
                dxc = ld("dxc", "dyc")
                pxc = tmp("p1")
                nc.vector.tensor_add(pxc[:], bxc[:], dxc[:])
                x0c = tmp("v2")
                floor_(pxc, x0c)
                txc = tmp("p4")
                nc.vector.tensor_scalar(txc[:], x0c[:], 1.0, 67.0,
                                        _alu("add"), _alu("min"))
                txc2 = tmp("p2")
                nc.vector.tensor_scalar(txc2[:], txc[:], 0.0, None,
                                        _alu("max"))
                idxf = tmp("p4")
                nc.vector.tensor_add(idxf[:], tyc2[:], txc2[:])
                idxc = tmp("ic0", I16)
                nc.vector.tensor_copy(idxc[:], idxf[:])

                # replicate wrapped idx to both core groups per stream
                nc.sync.dma_start(istage.ap(), idxc[:])
                for im in range(2):
                    for hf in range(2):
                        lo0 = im * 64 + hf * 32
                        ssrc = istage.ap()[lo0:lo0 + 32, :].rearrange(
                            "(s p) c -> p s c", s=2)
                        for rep in range(2):
                            lo = lo0 + rep * 16
                            dst = idxw[lo:lo + 16, :].rearrange(
                                "p (s c) -> p s c", s=2)
                            nc.sync.dma_start(dst, ssrc)

                # --- patch table: 2 image-row chunks x 4 shifted copies ---
                # (ACT does the jx=0 copies, DVE the jx=1 copies)
                for half_id, nm in enumerate(("img2a", "img2b")):
                    img = tp.tile([128, 33 * WIN * 2], F32, tag="img",
                                  name=f"img_{half_id}")
                    nc.sync.dma_start(img[:], din[nm].ap())
                    imgv = img[:].rearrange("p (h w cc) -> p h w cc",
                                            w=WIN, cc=2)
                    r0 = 33 * half_id
                    for jy in (0, 1):
                        for jx in (0, 1):
                            j = 2 * jx + jy
                            dst = t8[:, 1 - jy + r0:1 - jy + r0 + 33,
                                     1 - jx:1 - jx + 66,
                                     2 * j:2 * j + 2]
                            src = imgv[:, :, :, :]
                            if jx == 0:
                                nc.scalar.copy(dst, src)
                            else:
                                nc.vector.tensor_copy(dst, src)

                # --- weight pipeline (raster compact [128, KF]) ---
                byt = ld("byt", "byc")
                dyt = ld("dyt", "dyc")
                pyt = tmp("p1")
                nc.vector.tensor_add(pyt[:], byt[:], dyt[:])
                y0t = tmp("v1")
                floor_(pyt, y0t)
                fy = tmp("p2")
                nc.vector.tensor_sub(fy[:], pyt[:], y0t[:])
                Y0 = tmp("Y0")
                nc.vector.tensor_scalar(Y0[:], fy[:], -1.0, 1.0,
                                        _alu("mult"), _alu("add"))
                ta = tmp("p3")
                nc.vector.tensor_scalar(ta[:], y0t[:], 0.0, None,
                                        _alu("is_ge"))
                tb = tmp("p4")
                nc.vector.tensor_scalar(tb[:], y0t[:], 65.0, None,
                                        _alu("is_le"))
                nc.vector.tensor_mul(ta[:], ta[:], tb[:])  # vy0
                nc.vector.tensor_mul(Y0[:], Y0[:], ta[:])
                nc.vector.tensor_scalar(ta[:], y0t[:], -1.0, None,
                                        _alu("is_ge"))
                nc.vector.tensor_scalar(tb[:], y0t[:], 64.0, None,
                                        _alu("is_le"))
                nc.vector.tensor_mul(ta[:], ta[:], tb[:])  # vy1
                Y1 = tmp("Y1")
                nc.vector.tensor_mul(Y1[:], fy[:], ta[:])

                bxt = ld("bxt", "byc")
                dxt = ld("dxt", "dyc")
                pxt = tmp("p1")
                nc.vector.tensor_add(pxt[:], bxt[:], dxt[:])
                x0t = tmp("v2")
                floor_(pxt, x0t)
                fx = tmp("p2")
                nc.vector.tensor_sub(fx[:], pxt[:], x0t[:])
                X0 = tmp("X0")
                nc.vector.tensor_scalar(X0[:], fx[:], -1.0, 1.0,
                                        _alu("mult"), _alu("add"))
                nc.vector.tensor_scalar(ta[:], x0t[:], 0.0, None,
                                        _alu("is_ge"))
                nc.vector.tensor_scalar(tb[:], x0t[:], 65.0, None,
                                        _alu("is_le"))
                nc.vector.tensor_mul(ta[:], ta[:], tb[:])  # vx0
                nc.vector.tensor_mul(X0[:], X0[:], ta[:])
                nc.vector.tensor_scalar(ta[:], x0t[:], -1.0, None,
                                        _alu("is_ge"))
                nc.vector.tensor_scalar(tb[:], x0t[:], 64.0, None,
                                        _alu("is_le"))
                nc.vector.tensor_mul(ta[:], ta[:], tb[:])  # vx1
                X1 = tmp("X1")
                nc.vector.tensor_mul(X1[:], fx[:], ta[:])

                # products -> wcc [128, (k s64) j] bf16, j = 2*jx + jy
                wcc = tp.tile([128, 4 * KF], BF16, tag="wcc", name="wcc")
                wccv = wcc[:].rearrange("p (c j) -> p c j", j=4)
                nc.vector.tensor_mul(wccv[:, :, 0], Y0[:], X0[:])
                nc.vector.tensor_mul(wccv[:, :, 1], Y1[:], X0[:])
                nc.vector.tensor_mul(wccv[:, :, 2], Y0[:], X1[:])
                nc.vector.tensor_mul(wccv[:, :, 3], Y1[:], X1[:])

                # reshuffle via DRAM into wc64 rows (b16*4 + im*2 + half)
                nc.sync.dma_start(wstage.ap(), wcc[:])
                srcw = wstage.ap().rearrange(
                    "(im bh half b1 sh) (k sj) -> (im bh half b1) sh k sj",
                    im=2, bh=8, half=2, b1=2, sh=2, k=K)
                dstw = wc64[:].rearrange(
                    "v (k sh sj) -> v sh k sj", k=K, sh=2)
                for sh in range(2):
                    nc.sync.dma_start(dstw[:, sh, :, :], srcw[:, sh, :, :])

                # selector + conv weights, cast to bf16
                indf = tp.tile([64, 16 * 128], F32, tag="indf", name="indf")
                nc.sync.dma_start(indf[:], din["ind64"].ap())
                nc.scalar.copy(ind64[:], indf[:])
                for wh in range(4):
                    wmf = tp.tile([128, 2 * KF], F32, tag="wmf",
                                  name=f"wmf_{wh}")
                    nc.sync.dma_start(
                        wmf[:], din["wm8"].ap()[:, wh * 2 * KF:
                                                (wh + 1) * 2 * KF])
                    nc.scalar.copy(
                        wm8[:, wh * 2 * KF:(wh + 1) * 2 * KF], wmf[:])

            # ---------- phase 2: main loop ----------
            tabv = tab[:].rearrange("p (n d) -> p n d", d=8)
            idxwv = idxw[:].rearrange("q (s c) -> q s c", s=2)
            with tc.tile_pool(name="smul", bufs=3) as spool, \
                 tc.tile_pool(name="red1", bufs=3) as rpool, \
                 tc.tile_pool(name="red2", bufs=4) as r2pool, \
                 tc.tile_pool(name="outs", bufs=2) as opool, \
                 tc.tile_pool(name="wps", bufs=2, space="PSUM") as wps, \
                 tc.tile_pool(name="ops", bufs=2, space="PSUM") as ops_:
                for t in range(NCHUNK):
                    sb, soff = t // 4, (t % 4) * 16
                    out_ps2 = [
                        ops_.tile([64, CHUNK], F32, tag=f"ops{i}",
                                  name=f"ops{i}_{t}") for i in range(2)]
                    for k in range(K):
                        g = gpool.tile([128, 256, 8], BF16, tag="g",
                                       name=f"g_{t}_{k}")
                        nc.gpsimd.ap_gather(
                            g[:], tabv,
                            idxwv[:, sb, k * 64 + soff:k * 64 + soff + 16],
                            channels=128, num_elems=NE, d=8, num_idxs=256)
                        wp = wps.tile([128, 1024], F32, tag="wp",
                                      name=f"wp_{t}_{k}")
                        for q2 in range(2):
                            nc.tensor.matmul(
                                wp[:, q2 * 512:(q2 + 1) * 512],
                                ind64[:, (2 * t + q2) * 128:
                                      (2 * t + q2) * 128 + 128],
                                wc64[:, k * 512:(k + 1) * 512],
                                start=True, stop=True)
                        s = spool.tile([128, 2048], BF16, tag="s",
                                       name=f"s_{t}_{k}")
                        wpb = wp[:].rearrange("p (n j) -> p n j", j=4) \
                            .unsqueeze(3).broadcast_to([128, 256, 4, 2])
                        nc.vector.tensor_tensor(
                            s[:].rearrange("p (n j cc) -> p n j cc",
                                           j=4, cc=2),
                            g[:].rearrange("p n (j cc) -> p n j cc", cc=2),
                            wpb, _alu("mult"))
                        sv = s[:].rearrange("p (n j cc) -> p n j cc",
                                            j=4, cc=2)
                        r1 = rpool.tile([128, 256, 2, 2], BF16, tag="r1",
                                        name=f"r1_{t}_{k}")
                        nc.vector.tensor_add(r1[:], sv[:, :, 0:2, :],
                                             sv[:, :, 2:4, :])
                        r2 = r2pool.tile([128, 256, 2], BF16, tag="r2",
                                         name=f"r2_{t}_{k}")
                        nc.vector.tensor_add(r2[:], r1[:, :, 0, :],
                                             r1[:, :, 1, :])
                        for im in range(2):
                            for hf in range(2):
                                for cc in range(2):
                                    vi = im * 4 + hf * 2 + cc
                                    nc.tensor.matmul(
                                        out_ps2[im][:, hf * 256:
                                                    hf * 256 + 256],
                                        wm8[:, vi * KF + k * 64:
                                            vi * KF + k * 64 + 64],
                                        r2[:, :, cc],
                                        start=(k == 0 and cc == 0
                                               and hf == 0),
                                        stop=(k == 8 and cc == 1
                                              and hf == 1))
                    for im in range(2):
                        ot = opool.tile([64, CHUNK], F32, tag=f"ot{im}",
                                        name=f"ot{im}_{t}")
                        nc.scalar.copy(ot[:], out_ps2[im][:, :])
                        nc.sync.dma_start(
                            out_d.ap()[im * 64:(im + 1) * 64,
                                       t * CHUNK:(t + 1) * CHUNK], ot[:])

    nc.compile()
    return nc


# ---------------- host side ----------------

def _host_arrays(input, offset, weight):
    inp = np.ascontiguousarray(input, dtype=np.float32)
    off = np.ascontiguousarray(offset, dtype=np.float32)
    w = np.ascontiguousarray(weight, dtype=np.float32)

    wk = w.reshape(COUT, CIN, K)
    wcko = wk.transpose(1, 2, 0)  # [c, k, o]
    # wm8[p, vi*KF + k*64 + o] = w[o, 2*(p%32)+cc, k] masked to (im, half)
    p_ = np.arange(128)
    wm8 = np.zeros((128, 8 * KF), np.float32)
    for vi in range(8):
        im, hf, cc = vi // 4, (vi % 4) // 2, vi % 2
        mask = (p_ // 64 == im) & ((p_ % 64) // 32 == hf)
        rows = wcko[2 * (p_ % 32) + cc].reshape(128, KF)
        wm8[:, vi * KF:(vi + 1) * KF] = rows * mask[:, None]

    # ind64 rows v = im*32 + (b16//2)*4 + half*2 + (b16%2)
    # ind64[v, blk*128+p] = 1 iff b16==blk and p in (im, half)
    ind64 = np.zeros((64, 16, 128), np.float32)
    for v in range(64):
        im, bh, hf, b1 = v // 32, (v % 32) // 4, (v % 4) // 2, v % 2
        b16 = bh * 2 + b1
        ind64[v, b16] = (p_ // 64 == im) & ((p_ % 64) // 32 == hf)
    ind64 = ind64.reshape(64, 16 * 128)

    P = np.arange(NPX)
    ho = (P // WO).astype(np.float32)
    wo = (P % WO).astype(np.float32)
    kh = (np.arange(K) // KW).astype(np.float32)
    kw = (np.arange(K) % KW).astype(np.float32)

    u = np.arange(128)
    im_u = u // 64
    # stream-wrapped (idx pipeline): spx=(sb*64+cx)*16+pp of stream
    # (im, half); global P = (spx//256)*512 + half*256 + spx%256
    half_u = (u % 64) // 32
    sb_u = (u % 32) // 16
    pp_u = u % 16
    cx = np.arange(64)
    spx = (sb_u[:, None] * 64 + cx[None, :]) * 16 + pp_u[:, None]
    pix_c = (spx // 256) * 512 + half_u[:, None] * 256 + spx % 256
    # raster (weight pipeline): pixel = (u%64)*64 + s
    pix_t = (u % 64)[:, None] * 64 + cx[None, :]

    def expand(base_vals, tap_off, pix):
        b = base_vals[pix]  # [128, 64]
        return np.ascontiguousarray(
            (b[:, None, :] + tap_off[None, :, None]).reshape(128, KF))

    byc = expand(ho, kh, pix_c)
    bxc = expand(wo, kw, pix_c)
    byt = expand(ho, kh, pix_t)
    bxt = expand(wo, kw, pix_t)

    offr = off.reshape(N, K, 2, NPX)

    in_maps = []
    for core in range(NCORES):
        na, nb = 2 * core, 2 * core + 1
        # img2: partition (im, half, cpair) -> channels (2cp, 2cp+1)
        # interleaved [h, w, cc]; both halves carry the same channels
        im4 = np.empty((128, HIN, WIN, 2), np.float32)
        for im in range(2):
            src = inp[na if im == 0 else nb]  # [64, 66, 66]
            pair = src.reshape(32, 2, HIN, WIN).transpose(0, 2, 3, 1)
            im4[im * 64:im * 64 + 32] = pair
            im4[im * 64 + 32:(im + 1) * 64] = pair
        img2a = np.ascontiguousarray(im4[:, 0:33].reshape(128, -1))
        img2b = np.ascontiguousarray(im4[:, 33:66].reshape(128, -1))

        dy_ab = offr[[na, nb]][:, :, 0, :]  # [2, K, NPX]
        dx_ab = offr[[na, nb]][:, :, 1, :]

        def relay(arr, pix):
            g = arr[im_u[:, None], :, pix]  # [128, 64, K]
            return np.ascontiguousarray(
                g.transpose(0, 2, 1).reshape(128, KF))

        in_maps.append(dict(
            img2a=img2a, img2b=img2b, wm8=wm8, ind64=ind64,
            dyc=relay(dy_ab, pix_c), dxc=relay(dx_ab, pix_c),
            byc=byc, bxc=bxc,
            dyt=relay(dy_ab, pix_t), dxt=relay(dx_ab, pix_t),
            byt=byt, bxt=bxt,
        ))
    return in_maps


_NC_CACHE = None


def get_nc():
    global _NC_CACHE
    if _NC_CACHE is None:
        _NC_CACHE = build_bass()
    return _NC_CACHE


def kernel(input, offset, weight, _trace=False):
    from concourse.bass_utils import run_bass_kernel_spmd

    nc = get_nc()
    in_maps = _host_arrays(np.asarray(input), np.asarray(offset),
                           np.asarray(weight))
    res = run_bass_kernel_spmd(nc, in_maps, list(range(NCORES)), trace=_trace)
    out = np.empty((N, COUT, HO, WO), np.float32)
    for core in range(NCORES):
        o128 = np.asarray(res.results[core]["out128"])
        out[2 * core] = o128[0:64].reshape(COUT, HO, WO)
        out[2 * core + 1] = o128[64:128].reshape(COUT, HO, WO)
    if _trace:
        return out, res
    return out
